# revision 9
# baseline (speedup 1.0000x reference)
"""Trainium2 Bass kernel for nn_Algebraic_65970697666729 (segment_reduce).

Computes, for x of shape (131072, 16) fp32:
    out = concat([x, all C(16,2)=120 pairwise products, all C(16,3)=560
                  triple products], axis=1)  -> (131072, 696) fp32

Sharding: pure data parallel over rows; 8 cores x 16384 rows each.

Key design points (from ntff traces / the DVE microarch docs):
  * HBM traffic is minimized by storing the 680 product columns in bf16
    (rel-err ~3.5e-3 vs the 2e-2 gate) and skipping the 16 passthrough x
    columns entirely -- the host stitches the original fp32 x back in.
  * The DVE reaches its 2x packed mode (2 results/cycle @0.96GHz) only
    when every non-scalar operand has a 2-byte dtype and innermost AP dim
    [stride +-1, count >= 2].  In row-major layout the broadcast factor
    has innermost stride 0 -> locked to 1x.  So compute runs in a
    TRANSPOSED per-partition layout [cols, rows]: rows innermost for all
    three operands; the broadcast sits on the unchecked outer dim.  The
    host pre-transposes x and un-transposes the result.
  * The 16 HW DMA engines drain the output queue at ~330-350 GB/s, which
    makes the output stream (22.3 MB/core) the critical path.  To start
    it as early as possible the rows are processed in 3 blocks: after
    block 0's pairs (~4 us of DVE) the first section DMA is already in
    flight.  Output sections (pairs, then each triple run group) are
    DMA'd as they complete; each section lives in its own tile so
    section DMAs and later DVE writes share no tile (no false WAR deps).
  * HBM layouts are block-major so every section DMA is contiguous per
    partition; input blocks are prefetched on the scalar engine's DGE
    queue so they never sit behind output sections on the sync queue.

Compute (29 tensor_mul per block on the vector engine, one multiply per
output element):
  - pairs:   for i in 0..14:  P[po(i):...] = bcast(x_i) * x[i+1:16]
  - triples: for i in 0..13:  triples with first index i are exactly
             bcast(x_i) * (pairs with first index >= i+1), a contiguous
             suffix of the pairs tile.
"""

import numpy as np

N_CORES = 8
ROWS_TOTAL = 131072
ROWS = ROWS_TOTAL // N_CORES  # 16384
N = 16
NPAIRS = 120
NTRIPLES = 560
OUT_DEV = NPAIRS + NTRIPLES  # 680 product columns stored by the device
OUT_FULL = N + OUT_DEV  # 696
P = 128
R = ROWS // P  # 128 rows per partition

# Rows-per-partition per block (must be even for 4B-aligned bf16 runs).
# Tiny first block primes the output-DMA stream early; later blocks' pairs
# are issued mid-way through the previous block's triples so the DMA queue
# never starves while a pairs phase runs.
R_BLOCKS = [8, 60, 60]
assert sum(R_BLOCKS) == R and all(r % 2 == 0 for r in R_BLOCKS)

# Triple runs grouped per output DMA (ranges of the first index i).
TRI_GROUPS = [(0, 1), (1, 2), (2, 3), (3, 4), (4, 5), (5, 7), (7, 10), (10, 14)]

_CACHE = {}


def _pair_offsets():
    # po[i] = index (within the pairs section) of the first pair (i, *)
    po = [0] * (N + 1)
    for i in range(1, N + 1):
        po[i] = po[i - 1] + (N - 1 - (i - 1))
    return po


def _triple_offsets():
    # to[i] = index (within the triples section) of the first triple (i, *, *)
    to = [0] * (N - 1)
    for i in range(1, N - 1):
        m = N - 1 - (i - 1)  # suffix size after index i-1
        to[i] = to[i - 1] + m * (m - 1) // 2
    return to


def _build():
    import concourse.bacc as bacc
    import concourse.mybir as mybir
    from concourse import tile

    bf16 = mybir.dt.bfloat16
    nc = bacc.Bacc(
        "TRN2",
        target_bir_lowering=False,
        debug=False,
        enable_asserts=False,
        num_devices=N_CORES,
    )
    # Flat per-partition layouts, packed block-major by the host:
    #   xin[p, boff_x(q) + f*RQ + r] = x[p*128 + row0(q) + r, f]
    #   out[p, boff_o(q) + c*RQ + r] = product_col_c(row p*128 + row0(q) + r)
    xin = nc.dram_tensor("x", [P, N * R], bf16, kind="ExternalInput")
    out = nc.dram_tensor("out", [P, OUT_DEV * R], bf16, kind="ExternalOutput")

    po = _pair_offsets()
    to = _triple_offsets()
    to_end = to + [NTRIPLES]

    with tile.TileContext(nc) as tc:
        with tc.tile_pool(name="sp", bufs=1) as sp:
            xts, pts, gtss = [], [], []
            for q, RQ in enumerate(R_BLOCKS):
                xts.append(sp.tile([P, N, RQ], bf16, name=f"x{q}"))
                pts.append(sp.tile([P, NPAIRS, RQ], bf16, name=f"p{q}"))
                gtss.append(
                    [
                        sp.tile([P, to_end[b] - to[a], RQ], bf16, name=f"g{q}_{a}")
                        for a, b in TRI_GROUPS
                    ]
                )

            # Prefetch every block's x on the scalar engine's DGE queue so
            # the input never queues behind output sections.
            xoff = 0
            for q, RQ in enumerate(R_BLOCKS):
                src = xin.ap()[:, xoff : xoff + N * RQ].rearrange(
                    "p (f r) -> p f r", f=N
                )
                nc.scalar.dma_start(out=xts[q][:], in_=src)
                xoff += N * RQ

            ooffs = []
            o = 0
            for RQ in R_BLOCKS:
                ooffs.append(o)
                o += OUT_DEV * RQ

            def emit_pairs(q):
                RQ = R_BLOCKS[q]
                xT, pT = xts[q], pts[q]
                for i in range(N - 1):
                    L = N - 1 - i
                    a = po[i]
                    nc.vector.tensor_mul(
                        out=pT[:, a : a + L, :],
                        in0=xT[:, i + 1 : N, :],
                        in1=xT[:, i : i + 1, :].broadcast_to([P, L, RQ]),
                    )
                dst = out.ap()[:, ooffs[q] : ooffs[q] + NPAIRS * RQ].rearrange(
                    "p (c r) -> p c r", c=NPAIRS
                )
                nc.sync.dma_start(out=dst, in_=pT[:])

            def emit_tri_group(q, g):
                RQ = R_BLOCKS[q]
                xT, pT, gt = xts[q], pts[q], gtss[q][g]
                ia, ib = TRI_GROUPS[g]
                base = to[ia]
                for i in range(ia, ib):
                    m = N - 1 - i  # suffix size after i
                    L = m * (m - 1) // 2
                    a = to[i] - base
                    nc.vector.tensor_mul(
                        out=gt[:, a : a + L, :],
                        in0=pT[:, po[i + 1] : NPAIRS, :],
                        in1=xT[:, i : i + 1, :].broadcast_to([P, L, RQ]),
                    )
                ncols = to_end[ib] - to[ia]
                c0 = ooffs[q] + (NPAIRS + to[ia]) * RQ
                dst = out.ap()[:, c0 : c0 + ncols * RQ].rearrange(
                    "p (c r) -> p c r", c=ncols
                )
                nc.sync.dma_start(out=dst, in_=gt[:])

            NG = len(TRI_GROUPS)
            emit_pairs(0)
            for g in range(NG):
                emit_tri_group(0, g)
            emit_pairs(1)
            for g in range(3):
                emit_tri_group(1, g)
            emit_pairs(2)  # hidden inside block 1's triple phase
            for g in range(3, NG):
                emit_tri_group(1, g)
            for g in range(NG):
                emit_tri_group(2, g)

    nc.compile()
    return nc


def _run(x, trace=False, **spmd_kwargs):
    import ml_dtypes
    from concourse.bass_utils import run_bass_kernel_spmd

    if "nc" not in _CACHE:
        _CACHE["nc"] = _build()
    nc = _CACHE["nc"]

    x = np.ascontiguousarray(np.asarray(x, dtype=np.float32))
    assert x.shape == (ROWS_TOTAL, N), x.shape
    xb = x.astype(ml_dtypes.bfloat16)
    # [cores, P, R, N]
    x4 = xb.reshape(N_CORES, P, R, N)
    in_maps = []
    for i in range(N_CORES):
        packed = np.empty((P, N * R), dtype=ml_dtypes.bfloat16)
        r0 = 0
        off = 0
        for RQ in R_BLOCKS:
            blk = x4[i, :, r0 : r0 + RQ, :].transpose(0, 2, 1)  # [P, N, RQ]
            packed[:, off : off + N * RQ] = blk.reshape(P, N * RQ)
            r0 += RQ
            off += N * RQ
        in_maps.append({"x": packed})
    res = run_bass_kernel_spmd(
        nc, in_maps, core_ids=list(range(N_CORES)), trace=trace, **spmd_kwargs
    )
    full = np.empty((ROWS_TOTAL, OUT_FULL), dtype=np.float32)
    full[:, :N] = x
    prod = full[:, N:].reshape(N_CORES, P, R, OUT_DEV)
    for i, r in enumerate(res.results):
        dev = np.asarray(r["out"])  # [P, OUT_DEV * R] block-major
        r0 = 0
        off = 0
        for RQ in R_BLOCKS:
            blk = dev[:, off : off + OUT_DEV * RQ].reshape(P, OUT_DEV, RQ)
            prod[i, :, r0 : r0 + RQ, :] = blk.transpose(0, 2, 1).astype(np.float32)
            r0 += RQ
            off += OUT_DEV * RQ
    return full, res


def kernel(x):
    return _run(x)[0]


# revision 10
# speedup vs baseline: 1.0284x; 1.0284x over previous
"""Trainium2 Bass kernel for nn_Algebraic_65970697666729 (segment_reduce).

Computes, for x of shape (131072, 16) fp32:
    out = concat([x, all C(16,2)=120 pairwise products, all C(16,3)=560
                  triple products], axis=1)  -> (131072, 696) fp32

Sharding: pure data parallel over rows; 8 cores x 16384 rows each.

Key design points (from ntff traces / the DVE microarch docs):
  * HBM traffic is minimized by storing the 680 product columns in bf16
    (rel-err ~3.5e-3 vs the 2e-2 gate) and skipping the 16 passthrough x
    columns entirely -- the host stitches the original fp32 x back in.
  * The DVE reaches its 2x packed mode (2 results/cycle @0.96GHz) only
    when every non-scalar operand has a 2-byte dtype and innermost AP dim
    [stride +-1, count >= 2].  In row-major layout the broadcast factor
    has innermost stride 0 -> locked to 1x.  So compute runs in a
    TRANSPOSED per-partition layout [cols, rows]: rows innermost for all
    three operands; the broadcast sits on the unchecked outer dim.  The
    host pre-transposes x and un-transposes the result.
  * The 16 HW DMA engines drain the output queue at ~420 GB/s when fed,
    so the schedule is built to keep section supply ahead of the stream:
    rows are processed in 3 blocks; each block's pairs are split into two
    tiles (cols 0:65 / 65:120) so the first bytes ship early; the next
    block's pairs are issued in the middle of the current block's big
    triple groups; and each block's small triple groups (little data,
    much per-instruction overhead) are deferred into the next block's
    phase.  Every DMA'd section lives in its own tile, so section DMAs
    and later DVE writes never share a tile (no false WAR deps).

Compute (one multiply per output element, all on the vector engine):
  - pairs:   for i in 0..14:  P[po(i):...] = bcast(x_i) * x[i+1:16]
  - triples: for i in 0..13:  triples with first index i are exactly
             bcast(x_i) * (pairs with first index >= i+1), a contiguous
             suffix of the pairs section (split in two where it crosses
             the pA/pB tile boundary).
"""

import numpy as np

N_CORES = 8
ROWS_TOTAL = 131072
ROWS = ROWS_TOTAL // N_CORES  # 16384
N = 16
NPAIRS = 120
NTRIPLES = 560
OUT_DEV = NPAIRS + NTRIPLES  # 680 product columns stored by the device
OUT_FULL = N + OUT_DEV  # 696
P = 128
R = ROWS // P  # 128 rows per partition

# Rows-per-partition per block (must be even for 4B-aligned bf16 runs).
R_BLOCKS = [32, 48, 48]
assert sum(R_BLOCKS) == R and all(r % 2 == 0 for r in R_BLOCKS)

# Pairs split: pA holds pair runs i < PSPLIT (cols 0:65), pB the rest.
PSPLIT = 5
# Triple runs grouped per output DMA (ranges of the first index i).
# Groups 0..4 are "big" (shipped inline), 5..7 "small" (deferred).
TRI_GROUPS = [(0, 1), (1, 2), (2, 3), (3, 4), (4, 5), (5, 7), (7, 10), (10, 14)]
NBIG = 5

_CACHE = {}


def _pair_offsets():
    # po[i] = index (within the pairs section) of the first pair (i, *)
    po = [0] * (N + 1)
    for i in range(1, N + 1):
        po[i] = po[i - 1] + (N - 1 - (i - 1))
    return po


def _triple_offsets():
    # to[i] = index (within the triples section) of the first triple (i, *, *)
    to = [0] * (N - 1)
    for i in range(1, N - 1):
        m = N - 1 - (i - 1)  # suffix size after index i-1
        to[i] = to[i - 1] + m * (m - 1) // 2
    return to


def _build():
    import concourse.bacc as bacc
    import concourse.mybir as mybir
    from concourse import tile

    bf16 = mybir.dt.bfloat16
    nc = bacc.Bacc(
        "TRN2",
        target_bir_lowering=False,
        debug=False,
        enable_asserts=False,
        num_devices=N_CORES,
    )
    # Flat per-partition layouts, packed block-major by the host:
    #   xin[p, boff_x(q) + f*RQ + r] = x[p*128 + row0(q) + r, f]
    #   out[p, boff_o(q) + c*RQ + r] = product_col_c(row p*128 + row0(q) + r)
    xin = nc.dram_tensor("x", [P, N * R], bf16, kind="ExternalInput")
    out = nc.dram_tensor("out", [P, OUT_DEV * R], bf16, kind="ExternalOutput")

    po = _pair_offsets()
    to = _triple_offsets()
    to_end = to + [NTRIPLES]
    PA = po[PSPLIT]  # 65 cols in pA
    PB = NPAIRS - PA  # 55 cols in pB

    nb = len(R_BLOCKS)
    ooffs = []
    o = 0
    for RQ in R_BLOCKS:
        ooffs.append(o)
        o += OUT_DEV * RQ

    with tile.TileContext(nc) as tc:
        with tc.tile_pool(name="sp", bufs=1) as sp:
            xts, pas, pbs, gtss = [], [], [], []
            for q, RQ in enumerate(R_BLOCKS):
                xts.append(sp.tile([P, N, RQ], bf16, name=f"x{q}"))
                pas.append(sp.tile([P, PA, RQ], bf16, name=f"pa{q}"))
                pbs.append(sp.tile([P, PB, RQ], bf16, name=f"pb{q}"))
                gtss.append(
                    [
                        sp.tile([P, to_end[b] - to[a], RQ], bf16, name=f"g{q}_{a}")
                        for a, b in TRI_GROUPS
                    ]
                )

            # Prefetch every block's x on the scalar engine's DGE queue so
            # the input never queues behind output sections.
            xoff = 0
            for q, RQ in enumerate(R_BLOCKS):
                src = xin.ap()[:, xoff : xoff + N * RQ].rearrange(
                    "p (f r) -> p f r", f=N
                )
                nc.scalar.dma_start(out=xts[q][:], in_=src)
                xoff += N * RQ

            def dma_cols(q, c0, ncols, src_tile):
                RQ = R_BLOCKS[q]
                s = ooffs[q] + c0 * RQ
                dst = out.ap()[:, s : s + ncols * RQ].rearrange(
                    "p (c r) -> p c r", c=ncols
                )
                nc.sync.dma_start(out=dst, in_=src_tile[:])

            def emit_pairs(q):
                RQ = R_BLOCKS[q]
                xT = xts[q]
                for i in range(N - 1):
                    L = N - 1 - i
                    if i < PSPLIT:
                        dst = pas[q][:, po[i] : po[i] + L, :]
                    else:
                        dst = pbs[q][:, po[i] - PA : po[i] - PA + L, :]
                    nc.vector.tensor_mul(
                        out=dst,
                        in0=xT[:, i + 1 : N, :],
                        in1=xT[:, i : i + 1, :].broadcast_to([P, L, RQ]),
                    )
                    if i == PSPLIT - 1:
                        dma_cols(q, 0, PA, pas[q])
                dma_cols(q, PA, PB, pbs[q])

            def emit_tri_group(q, g):
                RQ = R_BLOCKS[q]
                xT, gt = xts[q], gtss[q][g]
                ia, ib = TRI_GROUPS[g]
                base = to[ia]
                for i in range(ia, ib):
                    m = N - 1 - i  # suffix size after i
                    L = m * (m - 1) // 2
                    a = to[i] - base
                    x1 = xT[:, i : i + 1, :]
                    if po[i + 1] < PA:
                        # pairs suffix crosses the pA/pB boundary: two muls
                        La = PA - po[i + 1]
                        nc.vector.tensor_mul(
                            out=gt[:, a : a + La, :],
                            in0=pas[q][:, po[i + 1] : PA, :],
                            in1=x1.broadcast_to([P, La, RQ]),
                        )
                        nc.vector.tensor_mul(
                            out=gt[:, a + La : a + L, :],
                            in0=pbs[q][:, 0:PB, :],
                            in1=x1.broadcast_to([P, PB, RQ]),
                        )
                    else:
                        nc.vector.tensor_mul(
                            out=gt[:, a : a + L, :],
                            in0=pbs[q][:, po[i + 1] - PA : PB, :],
                            in1=x1.broadcast_to([P, L, RQ]),
                        )
                dma_cols(q, NPAIRS + to[ia], to_end[ib] - to[ia], gt)

            # Schedule: pairs of block q+1 go out mid-way through block q's
            # big triple groups; small groups of block q are deferred into
            # block q+1's phase.
            emit_pairs(0)
            for q in range(nb):
                for g in range(NBIG):
                    emit_tri_group(q, g)
                    if g == 2 and q + 1 < nb:
                        emit_pairs(q + 1)
                if q > 0:
                    for g in range(NBIG, len(TRI_GROUPS)):
                        emit_tri_group(q - 1, g)
            for g in range(NBIG, len(TRI_GROUPS)):
                emit_tri_group(nb - 1, g)

    nc.compile()
    return nc


def _run(x, trace=False, **spmd_kwargs):
    import ml_dtypes
    from concourse.bass_utils import run_bass_kernel_spmd

    if "nc" not in _CACHE:
        _CACHE["nc"] = _build()
    nc = _CACHE["nc"]

    x = np.ascontiguousarray(np.asarray(x, dtype=np.float32))
    assert x.shape == (ROWS_TOTAL, N), x.shape
    xb = x.astype(ml_dtypes.bfloat16)
    # [cores, P, R, N]
    x4 = xb.reshape(N_CORES, P, R, N)
    in_maps = []
    for i in range(N_CORES):
        packed = np.empty((P, N * R), dtype=ml_dtypes.bfloat16)
        r0 = 0
        off = 0
        for RQ in R_BLOCKS:
            blk = x4[i, :, r0 : r0 + RQ, :].transpose(0, 2, 1)  # [P, N, RQ]
            packed[:, off : off + N * RQ] = blk.reshape(P, N * RQ)
            r0 += RQ
            off += N * RQ
        in_maps.append({"x": packed})
    res = run_bass_kernel_spmd(
        nc, in_maps, core_ids=list(range(N_CORES)), trace=trace, **spmd_kwargs
    )
    full = np.empty((ROWS_TOTAL, OUT_FULL), dtype=np.float32)
    full[:, :N] = x
    prod = full[:, N:].reshape(N_CORES, P, R, OUT_DEV)
    for i, r in enumerate(res.results):
        dev = np.asarray(r["out"])  # [P, OUT_DEV * R] block-major
        r0 = 0
        off = 0
        for RQ in R_BLOCKS:
            blk = dev[:, off : off + OUT_DEV * RQ].reshape(P, OUT_DEV, RQ)
            prod[i, :, r0 : r0 + RQ, :] = blk.transpose(0, 2, 1).astype(np.float32)
            r0 += RQ
            off += OUT_DEV * RQ
    return full, res


def kernel(x):
    return _run(x)[0]


# revision 12
# speedup vs baseline: 1.0357x; 1.0071x over previous
"""Trainium2 Bass kernel for nn_Algebraic_65970697666729 (segment_reduce).

Computes, for x of shape (131072, 16) fp32:
    out = concat([x, all C(16,2)=120 pairwise products, all C(16,3)=560
                  triple products], axis=1)  -> (131072, 696) fp32

Sharding: pure data parallel over rows; 8 cores x 16384 rows each.

Key design points (from ntff traces / the DVE microarch docs):
  * HBM traffic is minimized by storing the 680 product columns in bf16
    (rel-err ~3.5e-3 vs the 2e-2 gate) and skipping the 16 passthrough x
    columns entirely -- the host stitches the original fp32 x back in.
  * The DVE reaches its 2x packed mode (2 results/cycle @0.96GHz) only
    when every non-scalar operand has a 2-byte dtype and innermost AP dim
    [stride +-1, count >= 2].  In row-major layout the broadcast factor
    has innermost stride 0 -> locked to 1x.  So compute runs in a
    TRANSPOSED per-partition layout [cols, rows]: rows innermost for all
    three operands; the broadcast sits on the unchecked outer dim.  The
    host pre-transposes x and un-transposes the result.
  * The 16 HW DMA engines drain the output queue at ~420 GB/s when fed,
    so the schedule is built to keep section supply ahead of the stream:
    rows are processed in 3 blocks; each block's pairs are split into two
    tiles (cols 0:65 / 65:120) so the first bytes ship early; the next
    block's pairs are issued in the middle of the current block's big
    triple groups; and each block's small triple groups (little data,
    much per-instruction overhead) are deferred into the next block's
    phase.  Every DMA'd section lives in its own tile, so section DMAs
    and later DVE writes never share a tile (no false WAR deps).

Compute (one multiply per output element, all on the vector engine):
  - pairs:   for i in 0..14:  P[po(i):...] = bcast(x_i) * x[i+1:16]
  - triples: for i in 0..13:  triples with first index i are exactly
             bcast(x_i) * (pairs with first index >= i+1), a contiguous
             suffix of the pairs section (split in two where it crosses
             the pA/pB tile boundary).
"""

import numpy as np

N_CORES = 8
ROWS_TOTAL = 131072
ROWS = ROWS_TOTAL // N_CORES  # 16384
N = 16
NPAIRS = 120
NTRIPLES = 560
OUT_DEV = NPAIRS + NTRIPLES  # 680 product columns stored by the device
OUT_FULL = N + OUT_DEV  # 696
P = 128
R = ROWS // P  # 128 rows per partition

# Rows-per-partition per block (must be even for 4B-aligned bf16 runs).
# Two blocks minimize DVE per-instruction overhead (the DVE supply rate is
# only ~3% above the DMA drain rate, so every instruction's fixed cost
# pushes the stream end out); the second block's pairs are issued between
# the first block's big triple groups so the stream never starves.
R_BLOCKS = [64, 64]
assert sum(R_BLOCKS) == R and all(r % 2 == 0 for r in R_BLOCKS)

# Pairs split: pA holds pair runs i < PSPLIT (cols 0:65), pB the rest.
PSPLIT = 5
# Triple runs grouped per output DMA (ranges of the first index i).
# Groups 0..4 are "big" (shipped inline), 5..7 "small" (deferred).
TRI_GROUPS = [(0, 1), (1, 2), (2, 3), (3, 4), (4, 5), (5, 7), (7, 10), (10, 14)]
NBIG = 5

_CACHE = {}


def _pair_offsets():
    # po[i] = index (within the pairs section) of the first pair (i, *)
    po = [0] * (N + 1)
    for i in range(1, N + 1):
        po[i] = po[i - 1] + (N - 1 - (i - 1))
    return po


def _triple_offsets():
    # to[i] = index (within the triples section) of the first triple (i, *, *)
    to = [0] * (N - 1)
    for i in range(1, N - 1):
        m = N - 1 - (i - 1)  # suffix size after index i-1
        to[i] = to[i - 1] + m * (m - 1) // 2
    return to


def _build():
    import concourse.bacc as bacc
    import concourse.mybir as mybir
    from concourse import tile

    bf16 = mybir.dt.bfloat16
    nc = bacc.Bacc(
        "TRN2",
        target_bir_lowering=False,
        debug=False,
        enable_asserts=False,
        num_devices=N_CORES,
    )
    # Flat per-partition layouts, packed block-major by the host:
    #   xin[p, boff_x(q) + f*RQ + r] = x[p*128 + row0(q) + r, f]
    #   out[p, boff_o(q) + c*RQ + r] = product_col_c(row p*128 + row0(q) + r)
    xin = nc.dram_tensor("x", [P, N * R], bf16, kind="ExternalInput")
    out = nc.dram_tensor("out", [P, OUT_DEV * R], bf16, kind="ExternalOutput")

    po = _pair_offsets()
    to = _triple_offsets()
    to_end = to + [NTRIPLES]
    PA = po[PSPLIT]  # 65 cols in pA
    PB = NPAIRS - PA  # 55 cols in pB

    nb = len(R_BLOCKS)
    ooffs = []
    o = 0
    for RQ in R_BLOCKS:
        ooffs.append(o)
        o += OUT_DEV * RQ

    with tile.TileContext(nc) as tc:
        with tc.tile_pool(name="sp", bufs=1) as sp:
            xts, pas, pbs, gtss = [], [], [], []
            for q, RQ in enumerate(R_BLOCKS):
                xts.append(sp.tile([P, N, RQ], bf16, name=f"x{q}"))
                pas.append(sp.tile([P, PA, RQ], bf16, name=f"pa{q}"))
                pbs.append(sp.tile([P, PB, RQ], bf16, name=f"pb{q}"))
                gtss.append(
                    [
                        sp.tile([P, to_end[b] - to[a], RQ], bf16, name=f"g{q}_{a}")
                        for a, b in TRI_GROUPS
                    ]
                )

            # Prefetch every block's x on the scalar engine's DGE queue so
            # the input never queues behind output sections.
            xoff = 0
            for q, RQ in enumerate(R_BLOCKS):
                src = xin.ap()[:, xoff : xoff + N * RQ].rearrange(
                    "p (f r) -> p f r", f=N
                )
                nc.scalar.dma_start(out=xts[q][:], in_=src)
                xoff += N * RQ

            def dma_cols(q, c0, ncols, src_tile):
                RQ = R_BLOCKS[q]
                s = ooffs[q] + c0 * RQ
                dst = out.ap()[:, s : s + ncols * RQ].rearrange(
                    "p (c r) -> p c r", c=ncols
                )
                nc.sync.dma_start(out=dst, in_=src_tile[:])

            def emit_pairs(q):
                RQ = R_BLOCKS[q]
                xT = xts[q]
                for i in range(N - 1):
                    L = N - 1 - i
                    if i < PSPLIT:
                        dst = pas[q][:, po[i] : po[i] + L, :]
                    else:
                        dst = pbs[q][:, po[i] - PA : po[i] - PA + L, :]
                    nc.vector.tensor_mul(
                        out=dst,
                        in0=xT[:, i + 1 : N, :],
                        in1=xT[:, i : i + 1, :].broadcast_to([P, L, RQ]),
                    )
                    if i == PSPLIT - 1:
                        dma_cols(q, 0, PA, pas[q])
                dma_cols(q, PA, PB, pbs[q])

            def emit_tri_group(q, g):
                RQ = R_BLOCKS[q]
                xT, gt = xts[q], gtss[q][g]
                ia, ib = TRI_GROUPS[g]
                base = to[ia]
                for i in range(ia, ib):
                    m = N - 1 - i  # suffix size after i
                    L = m * (m - 1) // 2
                    a = to[i] - base
                    x1 = xT[:, i : i + 1, :]
                    if po[i + 1] < PA:
                        # pairs suffix crosses the pA/pB boundary: two muls
                        La = PA - po[i + 1]
                        nc.vector.tensor_mul(
                            out=gt[:, a : a + La, :],
                            in0=pas[q][:, po[i + 1] : PA, :],
                            in1=x1.broadcast_to([P, La, RQ]),
                        )
                        nc.vector.tensor_mul(
                            out=gt[:, a + La : a + L, :],
                            in0=pbs[q][:, 0:PB, :],
                            in1=x1.broadcast_to([P, PB, RQ]),
                        )
                    else:
                        nc.vector.tensor_mul(
                            out=gt[:, a : a + L, :],
                            in0=pbs[q][:, po[i + 1] - PA : PB, :],
                            in1=x1.broadcast_to([P, L, RQ]),
                        )
                dma_cols(q, NPAIRS + to[ia], to_end[ib] - to[ia], gt)

            # Schedule: pairs of block q+1 go out mid-way through block q's
            # big triple groups; small groups of block q are deferred into
            # block q+1's phase.
            emit_pairs(0)
            for q in range(nb):
                for g in range(NBIG):
                    emit_tri_group(q, g)
                    if g == 1 and q + 1 < nb:
                        emit_pairs(q + 1)
                if q > 0:
                    for g in range(NBIG, len(TRI_GROUPS)):
                        emit_tri_group(q - 1, g)
            for g in range(NBIG, len(TRI_GROUPS)):
                emit_tri_group(nb - 1, g)

    nc.compile()
    return nc


def _run(x, trace=False, **spmd_kwargs):
    import ml_dtypes
    from concourse.bass_utils import run_bass_kernel_spmd

    if "nc" not in _CACHE:
        _CACHE["nc"] = _build()
    nc = _CACHE["nc"]

    x = np.ascontiguousarray(np.asarray(x, dtype=np.float32))
    assert x.shape == (ROWS_TOTAL, N), x.shape
    xb = x.astype(ml_dtypes.bfloat16)
    # [cores, P, R, N]
    x4 = xb.reshape(N_CORES, P, R, N)
    in_maps = []
    for i in range(N_CORES):
        packed = np.empty((P, N * R), dtype=ml_dtypes.bfloat16)
        r0 = 0
        off = 0
        for RQ in R_BLOCKS:
            blk = x4[i, :, r0 : r0 + RQ, :].transpose(0, 2, 1)  # [P, N, RQ]
            packed[:, off : off + N * RQ] = blk.reshape(P, N * RQ)
            r0 += RQ
            off += N * RQ
        in_maps.append({"x": packed})
    res = run_bass_kernel_spmd(
        nc, in_maps, core_ids=list(range(N_CORES)), trace=trace, **spmd_kwargs
    )
    full = np.empty((ROWS_TOTAL, OUT_FULL), dtype=np.float32)
    full[:, :N] = x
    prod = full[:, N:].reshape(N_CORES, P, R, OUT_DEV)
    for i, r in enumerate(res.results):
        dev = np.asarray(r["out"])  # [P, OUT_DEV * R] block-major
        r0 = 0
        off = 0
        for RQ in R_BLOCKS:
            blk = dev[:, off : off + OUT_DEV * RQ].reshape(P, OUT_DEV, RQ)
            prod[i, :, r0 : r0 + RQ, :] = blk.transpose(0, 2, 1).astype(np.float32)
            r0 += RQ
            off += OUT_DEV * RQ
    return full, res


def kernel(x):
    return _run(x)[0]
